# revision 9
# baseline (speedup 1.0000x reference)
"""Trainium2 Bass kernel for BailingMoeV2.5 linear attention (GLA) layer.

Sharding: TP=2 over heads x DP=4 over packed sequences -> 8 NeuronCores.
Each core processes 1 sequence (4096 tokens) x 8 heads. The dense output
projection is row-sharded over heads, so the host sums the two TP partials.

Math notes (exact reformulation of the reference chunked GLA, chunk C=128):
  h_t = e^g h_{t-1} + k_t v_t^T ;  o_t = SCALE * q_t h_t
  Within a chunk (i = position in chunk, 0-based):
    o_i = SCALE*e^{g(i+1)} q_i h0  +  SCALE*sum_{j<=i} e^{g(i-j)} (q_i.k_j) v_j
  We fold SCALE*e^{g(i+1)} into q~ at creation. Then
    o_i = q~_i h0 + sum_j M[j,i] (q~_i.k_j) v_j ,  M[j,i] = e^{-g(j+1)} 1[i>=j]
  State update: h' = e^{gC} h0 + sum_j e^{g(C-1-j)} k_j v_j.
"""

import math
import sys

sys.path.insert(0, "/opt/trn_rl_repo")

import numpy as np
import ml_dtypes

# ---- static config (must match the problem) ----
B, S, HID, H, D = 4, 4096, 2048, 16, 128
T = B * S
LAYER_IDX, N_LAYERS = 8, 32
ROT, HALF = 64, 32
THETA = 10000.0
EPS = 1e-6
GROUPS = 8
SCALE = D ** -0.5
C = 128              # GLA chunk size used on device (exact for any chunk size)
NCH = S // C         # 32 chunks per sequence
TPD = 2              # tensor parallel over heads
NH = H // TPD        # heads per core = 8
NCORES = 8

BF16 = ml_dtypes.bfloat16


def _build_slopes(n):
    def p2(n):
        start = 2.0 ** (-(2.0 ** (-(math.log2(n) - 3))))
        return [start * start ** i for i in range(n)]
    if math.log2(n).is_integer():
        return p2(n)
    c = 2 ** math.floor(math.log2(n))
    return p2(c) + _build_slopes(2 * c)[0::2][: n - c]


def _gamma_vec():
    base = np.array(_build_slopes(H), dtype=np.float64)
    decay = 1.0 - (LAYER_IDX - 1) / (N_LAYERS - 1) + 1e-5
    return (-base * decay).astype(np.float64)  # [H], negative


_PROG_CACHE = {}
TRACE = False
LAST_EXEC_NS = None


def _build_program(apply_qw, apply_kw):
    """Build the (shared, SPMD) Bass program for one core's shard."""
    import concourse.bass as bass
    import concourse.bacc as bacc
    import concourse.tile as tile
    from concourse import mybir

    f32 = mybir.dt.float32
    bf16 = mybir.dt.bfloat16
    MUL = mybir.AluOpType.mult

    nc = bacc.Bacc("TRN2", target_bir_lowering=False, debug=False)

    # ---------------- DRAM I/O ----------------
    # xt: hidden^T, chunk-major: [chunk, ki, ko, t], k = ko*128 + ki
    xt_d = nc.dram_tensor("xt", [NCH, 128, HID // 128, C], bf16, kind="ExternalInput")
    # w: fused [q|k|v|gate] weights, kxm layout [ki, ko, 4*NH*D]
    w_d = nc.dram_tensor("w", [128, HID // 128, 4 * NH * D], bf16, kind="ExternalInput")
    # wd: dense weights (g_norm_w folded), [ki, co, HID] over this core's NH*D rows
    wd_d = nc.dram_tensor("wd", [128, NH * D // 128, HID], bf16, kind="ExternalInput")
    cos_d = nc.dram_tensor("cosT", [S, HALF], f32, kind="ExternalInput")
    sin_d = nc.dram_tensor("sinT", [S, HALF], f32, kind="ExternalInput")
    dqs_d = nc.dram_tensor("dqs", [C, NH], f32, kind="ExternalInput")   # e^{g(i+1)}*SCALE
    dkv_d = nc.dram_tensor("dkv", [C, NH], f32, kind="ExternalInput")   # e^{g(C-1-i)}
    dcc_d = nc.dram_tensor("dcc", [NH], f32, kind="ExternalInput")      # e^{gC}
    mt_d = nc.dram_tensor("mt", [C, NH, C], f32, kind="ExternalInput")  # M[j,h,i]
    h0_d = nc.dram_tensor("h0", [128, NH, D], f32, kind="ExternalInput")  # [d, h, e]
    qw_d = nc.dram_tensor("qwv", [D], f32, kind="ExternalInput")
    kw_d = nc.dram_tensor("kwv", [D], f32, kind="ExternalInput")

    out_d = nc.dram_tensor("out", [S, HID], f32, kind="ExternalOutput")
    ht_d = nc.dram_tensor("ht", [128, NH, D], f32, kind="ExternalOutput")

    KO = HID // 128       # 16 k-subtiles
    NCOL = 4 * NH * D     # 4096 fused output cols
    NT1 = NCOL // 512     # 8 psum n-tiles in pass 1
    CO = NH * D // 128    # 8 contraction subtiles in dense
    NT2 = HID // 512      # 4 psum n-tiles in dense

    with tile.TileContext(nc) as tc:
        # DRAM scratch (tracked by Tile for cross-pass deps)
        with tc.tile_pool(name="dram", bufs=1, space="DRAM") as dram, \
             tc.tile_pool(name="gconst", bufs=1) as gconst:
            eps_t = gconst.tile([128, 1], f32)
            nc.vector.memset(eps_t[:], EPS)
            qb_d = dram.tile([NCH, C, NH, D], bf16, name="qb")
            kb_d = dram.tile([NCH, C, NH, D], bf16, name="kb")
            vb_d = dram.tile([NCH, C, NH, D], bf16, name="vb")
            gb_d = dram.tile([NCH, C, NH * D], bf16, name="gb")

            # =================== PASS 1 ===================
            with (
                tc.tile_pool(name="wpool", bufs=1) as wpool,
                tc.tile_pool(name="c1", bufs=1) as c1,
                tc.tile_pool(name="p1a", bufs=3) as p1a,
                tc.tile_pool(name="p1b", bufs=2) as p1b,
                tc.tile_pool(name="ps1", bufs=4, space="PSUM") as ps1,
            ):
                w_sb = wpool.tile([128, KO, NCOL], bf16)
                # split the 16.8MB weight load into KO DMAs for parallelism
                for k in range(KO):
                    nc.sync.dma_start(w_sb[:, k, :], w_d[:, k, :])
                dqs_sb = c1.tile([C, NH], f32)
                nc.sync.dma_start(dqs_sb[:], dqs_d[:])
                if apply_qw:
                    qw_sb = c1.tile([128, D], f32)
                    nc.gpsimd.dma_start(
                        qw_sb[:],
                        bass.AP(tensor=qw_d, offset=0, ap=[[0, 128], [1, D]]),
                    )
                if apply_kw:
                    kw_sb = c1.tile([128, D], f32)
                    nc.gpsimd.dma_start(
                        kw_sb[:],
                        bass.AP(tensor=kw_d, offset=0, ap=[[0, 128], [1, D]]),
                    )

                for c in range(NCH):
                    xt = p1a.tile([128, KO, C], bf16, tag="xt")
                    nc.sync.dma_start(xt[:], xt_d[c])
                    cos_t = p1a.tile([C, HALF], f32, tag="cos")
                    nc.sync.dma_start(cos_t[:], cos_d[c * C:(c + 1) * C, :])
                    sin_t = p1a.tile([C, HALF], f32, tag="sin")
                    nc.sync.dma_start(sin_t[:], sin_d[c * C:(c + 1) * C, :])

                    qk_sb = p1b.tile([C, 2 * NH * D], f32, tag="qk")
                    vb_t = p1a.tile([C, NH, D], bf16, tag="vb")
                    gb_t = p1a.tile([C, NH * D], bf16, tag="gb")

                    for n in range(NT1):
                        ps = ps1.tile([128, 512], f32, tag="ps")
                        for k in range(KO):
                            nc.tensor.matmul(
                                ps[:],
                                xt[:, k, :],
                                w_sb[:, k, n * 512:(n + 1) * 512],
                                start=(k == 0),
                                stop=(k == KO - 1),
                            )
                        if n < 4:  # q then k columns
                            nc.vector.tensor_copy(
                                qk_sb[:, n * 512:(n + 1) * 512], ps[:]
                            )
                        elif n < 6:  # v columns -> bf16 natural
                            off = (n - 4) * 512
                            nc.vector.tensor_copy(
                                vb_t.rearrange("p h d -> p (h d)")[
                                    :, off:off + 512
                                ],
                                ps[:],
                            )
                        else:  # gate columns -> sigmoid -> bf16
                            off = (n - 6) * 512
                            nc.scalar.activation(
                                gb_t[:, off:off + 512],
                                ps[:],
                                mybir.ActivationFunctionType.Sigmoid,
                            )

                    # ---- RMS norm stats over each head's 128 dims ----
                    # (q*1)*q per head with accum_out -> per-head sum of squares,
                    # without materializing a full q^2 buffer (SBUF pressure).
                    ss = p1a.tile([C, 2 * NH], f32, tag="ss")
                    trash = p1a.tile([C, D], f32, tag="trash")
                    qkv3 = qk_sb.rearrange("p (g d) -> p g d", d=D)
                    for hh in range(2 * NH):
                        nc.vector.scalar_tensor_tensor(
                            trash[:], qkv3[:, hh, :], 1.0, qkv3[:, hh, :],
                            MUL, MUL, accum_out=ss[:, hh:hh + 1],
                        )
                    rs = p1a.tile([C, 2 * NH], f32, tag="rs")
                    nc.scalar.activation(
                        rs[:], ss[:], mybir.ActivationFunctionType.Sqrt,
                        bias=eps_t[:], scale=1.0 / D,
                    )
                    nc.vector.reciprocal(rs[:], rs[:])
                    rq = p1a.tile([C, NH], f32, tag="rq")
                    nc.vector.tensor_mul(rq[:], rs[:, 0:NH], dqs_sb[:])

                    qv = qk_sb.rearrange("p (g d) -> p g d", d=D)
                    # q *= rms_r * e^{g(i+1)} * SCALE ; k *= rms_r
                    nc.vector.tensor_tensor(
                        qv[:, 0:NH, :], qv[:, 0:NH, :],
                        rq[:, :, None].to_broadcast([C, NH, D]), MUL,
                    )
                    nc.vector.tensor_tensor(
                        qv[:, NH:2 * NH, :], qv[:, NH:2 * NH, :],
                        rs[:, NH:2 * NH, None].to_broadcast([C, NH, D]), MUL,
                    )
                    if apply_qw:
                        nc.vector.tensor_tensor(
                            qv[:, 0:NH, :], qv[:, 0:NH, :],
                            qw_sb[:, None, :].to_broadcast([C, NH, D]), MUL,
                        )
                    if apply_kw:
                        nc.vector.tensor_tensor(
                            qv[:, NH:2 * NH, :], qv[:, NH:2 * NH, :],
                            kw_sb[:, None, :].to_broadcast([C, NH, D]), MUL,
                        )

                    # ---- partial RoPE (first 64 dims of each head) ----
                    cosb = cos_t[:, None, :].to_broadcast([C, NH, HALF])
                    sinb = sin_t[:, None, :].to_broadcast([C, NH, HALF])
                    qb_t = p1a.tile([C, NH, D], bf16, tag="qb")
                    kb_t = p1a.tile([C, NH, D], bf16, tag="kb")
                    for (src_off, dst) in ((0, qb_t), (NH, kb_t)):
                        x1 = qv[:, src_off:src_off + NH, 0:HALF]
                        x2 = qv[:, src_off:src_off + NH, HALF:ROT]
                        t1 = p1a.tile([C, NH, HALF], f32, tag="t1")
                        t2 = p1a.tile([C, NH, HALF], f32, tag="t2")
                        nc.vector.tensor_tensor(t1[:], x1, cosb, MUL)
                        nc.vector.tensor_tensor(t2[:], x2, sinb, MUL)
                        nc.vector.tensor_sub(dst[:, :, 0:HALF], t1[:], t2[:])
                        t3 = p1a.tile([C, NH, HALF], f32, tag="t1")
                        t4 = p1a.tile([C, NH, HALF], f32, tag="t2")
                        nc.vector.tensor_tensor(t3[:], x2, cosb, MUL)
                        nc.vector.tensor_tensor(t4[:], x1, sinb, MUL)
                        nc.vector.tensor_add(dst[:, :, HALF:ROT], t3[:], t4[:])
                        nc.vector.tensor_copy(
                            dst[:, :, ROT:D], qv[:, src_off:src_off + NH, ROT:D]
                        )

                    nc.sync.dma_start(qb_d[c], qb_t[:])
                    nc.sync.dma_start(kb_d[c], kb_t[:])
                    nc.sync.dma_start(vb_d[c], vb_t[:])
                    nc.sync.dma_start(gb_d[c], gb_t[:])

            # =================== PASS 2 ===================
            with (
                tc.tile_pool(name="c2", bufs=1) as c2,
                tc.tile_pool(name="p2a", bufs=3) as p2a,
                tc.tile_pool(name="p2b", bufs=2) as p2b,
                tc.tile_pool(name="pss", bufs=2, space="PSUM") as pss,
                tc.tile_pool(name="pso", bufs=2, space="PSUM") as pso,
                tc.tile_pool(name="psu", bufs=2, space="PSUM") as psu,
                tc.tile_pool(name="psd", bufs=2, space="PSUM") as psd,
            ):
                wd_sb = c2.tile([128, CO, HID], bf16)
                for k in range(CO):
                    nc.sync.dma_start(wd_sb[:, k, :], wd_d[:, k, :])
                mt_sb = c2.tile([C, NH, C], f32)
                nc.sync.dma_start(mt_sb[:], mt_d[:])
                dkv_sb = c2.tile([C, NH], f32)
                nc.sync.dma_start(dkv_sb[:], dkv_d[:])
                dcc_sb = c2.tile([128, NH], f32)
                nc.gpsimd.dma_start(
                    dcc_sb[:], bass.AP(tensor=dcc_d, offset=0, ap=[[0, 128], [1, NH]])
                )
                hf = []
                hb = []
                for h in range(NH):
                    hfh = c2.tile([128, D], f32, name=f"hf{h}")
                    nc.sync.dma_start(hfh[:], h0_d[:, h, :])
                    hbh = c2.tile([128, D], bf16, name=f"hb{h}")
                    nc.scalar.copy(hbh[:], hfh[:])
                    hf.append(hfh)
                    hb.append(hbh)

                for c in range(NCH):
                    ktn = p2a.tile([C, NH, D], bf16, tag="ktn")
                    nc.sync.dma_start(ktn[:], kb_d[c])
                    vn = p2a.tile([C, NH, D], bf16, tag="vn")
                    nc.sync.dma_start(vn[:], vb_d[c])
                    g_t = p2a.tile([C, NH * D], bf16, tag="g")
                    nc.sync.dma_start(g_t[:], gb_d[c])
                    qT = p2a.tile([128, NH, C], bf16, tag="qT")
                    kT = p2a.tile([128, NH, C], bf16, tag="kT")
                    for h in range(NH):
                        nc.sync.dma_start(
                            qT[:, h, :], qb_d[c, :, h, :], transpose=True
                        )
                        nc.sync.dma_start(
                            kT[:, h, :], kb_d[c, :, h, :], transpose=True
                        )

                    o_sb = p2b.tile([C, NH, D], f32, tag="o")
                    for h in range(NH):
                        ps_s = pss.tile([C, C], f32, tag="s")
                        nc.tensor.matmul(
                            ps_s[:], kT[:, h, :], qT[:, h, :], start=True, stop=True
                        )
                        sT = p2a.tile([C, C], bf16, tag="sT")
                        nc.vector.tensor_tensor(sT[:], ps_s[:], mt_sb[:, h, :], MUL)
                        ps_o = pso.tile([C, D], f32, tag="po")
                        nc.tensor.matmul(
                            ps_o[:], sT[:], vn[:, h, :], start=True, stop=False
                        )
                        nc.tensor.matmul(
                            ps_o[:], qT[:, h, :], hb[h][:], start=False, stop=True
                        )
                        nc.any.tensor_copy(o_sb[:, h, :], ps_o[:])
                        ksc = p2a.tile([C, D], bf16, tag="ksc")
                        nc.vector.tensor_scalar_mul(
                            ksc[:], ktn[:, h, :], dkv_sb[:, h:h + 1]
                        )
                        ps_u = psu.tile([D, D], f32, tag="pu")
                        nc.tensor.matmul(
                            ps_u[:], ksc[:], vn[:, h, :], start=True, stop=True
                        )
                        nc.vector.scalar_tensor_tensor(
                            hf[h][:], hf[h][:], dcc_sb[:, h:h + 1], ps_u[:],
                            mybir.AluOpType.mult, mybir.AluOpType.add,
                        )
                        nc.scalar.copy(hb[h][:], hf[h][:])

                    # ---- group RMS norm (groups of 256 = 2 heads) ----
                    of = o_sb.rearrange("p h d -> p (h d)")
                    gsq = p2b.tile([C, NH * D], f32, tag="gsq")
                    nc.vector.tensor_mul(gsq[:], of, of)
                    gss = p2a.tile([C, 4], f32, tag="gss")
                    nc.vector.reduce_sum(
                        gss[:],
                        gsq.rearrange("p (g d) -> p g d", g=4),
                        axis=mybir.AxisListType.X,
                    )
                    nc.scalar.activation(
                        gss[:], gss[:], mybir.ActivationFunctionType.Sqrt,
                        bias=eps_t[:], scale=1.0 / 256.0,
                    )
                    nc.vector.reciprocal(gss[:], gss[:])
                    on = p2b.tile([C, 4, 256], f32, tag="on")
                    nc.vector.tensor_tensor(
                        on[:],
                        o_sb.rearrange("p h d -> p (h d)").rearrange(
                            "p (g d) -> p g d", g=4
                        ),
                        gss[:, :, None].to_broadcast([C, 4, 256]),
                        MUL,
                    )
                    og = p2a.tile([C, NH * D], bf16, tag="og")
                    nc.vector.tensor_mul(
                        og[:], on.rearrange("p g d -> p (g d)"), g_t[:]
                    )

                    ogT = p2a.tile([128, CO, C], bf16, tag="ogT")
                    for h in range(CO):
                        nc.sync.dma_start(
                            ogT[:, h, :], og[:, h * 128:(h + 1) * 128],
                            transpose=True,
                        )

                    out_t = p2b.tile([C, HID], f32, tag="out")
                    for n in range(NT2):
                        ps_d = psd.tile([128, 512], f32, tag="pd")
                        for kk in range(CO):
                            nc.tensor.matmul(
                                ps_d[:],
                                ogT[:, kk, :],
                                wd_sb[:, kk, n * 512:(n + 1) * 512],
                                start=(kk == 0),
                                stop=(kk == CO - 1),
                            )
                        nc.any.tensor_copy(out_t[:, n * 512:(n + 1) * 512], ps_d[:])
                    nc.sync.dma_start(out_d[c * C:(c + 1) * C, :], out_t[:])

                for h in range(NH):
                    nc.sync.dma_start(ht_d[:, h, :], hf[h][:])

    nc.compile()
    return nc


def _get_program(apply_qw, apply_kw):
    key = (apply_qw, apply_kw)
    if key not in _PROG_CACHE:
        _PROG_CACHE[key] = _build_program(apply_qw, apply_kw)
    return _PROG_CACHE[key]


def _make_in_maps(positions, hidden_states, recurrent_state, w_qkv, w_g,
                  w_dense, q_norm_w, k_norm_w, g_norm_w):
    positions = np.asarray(positions)
    hidden_states = np.ascontiguousarray(np.asarray(hidden_states, dtype=np.float32))
    recurrent_state = np.asarray(recurrent_state, dtype=np.float32)
    w_qkv = np.asarray(w_qkv, dtype=np.float32)
    w_g = np.asarray(w_g, dtype=np.float32)
    w_dense = np.asarray(w_dense, dtype=np.float32)
    q_norm_w = np.asarray(q_norm_w, dtype=np.float32)
    k_norm_w = np.asarray(k_norm_w, dtype=np.float32)
    g_norm_w = np.asarray(g_norm_w, dtype=np.float32)

    apply_qw = not np.allclose(q_norm_w, 1.0)
    apply_kw = not np.allclose(k_norm_w, 1.0)

    gamma = _gamma_vec()  # [16] float64
    inv_freq = 1.0 / (THETA ** (np.arange(0, ROT, 2, dtype=np.float32) / ROT))
    ii = np.arange(C, dtype=np.float64)

    wq4 = w_qkv.reshape(HID, 3, H, D)
    wdg = (g_norm_w[:, None] * w_dense)  # [H*D, HID]

    in_maps = []
    for b in range(B):
        hb = hidden_states[b * S:(b + 1) * S]  # [S, HID]
        xt = np.ascontiguousarray(
            hb.reshape(NCH, C, HID // 128, 128).transpose(0, 3, 2, 1)
        ).astype(BF16)  # [NCH, ki, ko, t]
        pos_b = positions[b * S:(b + 1) * S].astype(np.float32)
        ang = pos_b[:, None] * inv_freq[None, :]
        cosb = np.cos(ang).astype(np.float32)
        sinb = np.sin(ang).astype(np.float32)
        for tp in range(TPD):
            hs0 = tp * NH
            hsl = slice(hs0, hs0 + NH)
            g_loc = gamma[hsl]  # [NH]

            Wcat = np.concatenate(
                [
                    wq4[:, 0, hsl, :].reshape(HID, NH * D),
                    wq4[:, 1, hsl, :].reshape(HID, NH * D),
                    wq4[:, 2, hsl, :].reshape(HID, NH * D),
                    w_g[:, hs0 * D:(hs0 + NH) * D],
                ],
                axis=1,
            )  # [HID, 4*NH*D]
            Wk = np.ascontiguousarray(
                Wcat.reshape(HID // 128, 128, 4 * NH * D).transpose(1, 0, 2)
            ).astype(BF16)
            wd = np.ascontiguousarray(
                wdg[hs0 * D:(hs0 + NH) * D, :]
                .reshape(NH, 128, HID).transpose(1, 0, 2)
            ).astype(BF16)  # [ki, co, HID]

            dqs = (np.exp(g_loc[None, :] * (ii[:, None] + 1.0)) * SCALE).astype(
                np.float32
            )  # [C, NH]
            dkv = np.exp(g_loc[None, :] * (C - 1.0 - ii[:, None])).astype(np.float32)
            dcc = np.exp(g_loc * C).astype(np.float32)  # [NH]
            # M[j, h, i] = e^{-g_h (j+1)} for i>=j else 0
            mt = np.where(
                ii[None, None, :] >= ii[:, None, None],
                np.exp(-g_loc[None, :, None] * (ii[:, None, None] + 1.0)),
                0.0,
            ).astype(np.float32)  # [C, NH, C]
            h0 = np.ascontiguousarray(
                recurrent_state[b, hsl].transpose(1, 0, 2)
            ).astype(np.float32)  # [d, h, e]

            in_maps.append(
                {
                    "xt": xt,
                    "w": Wk,
                    "wd": wd,
                    "cosT": cosb,
                    "sinT": sinb,
                    "dqs": dqs,
                    "dkv": dkv,
                    "dcc": dcc,
                    "mt": mt,
                    "h0": h0,
                    "qwv": q_norm_w,
                    "kwv": k_norm_w,
                }
            )

    return in_maps, apply_qw, apply_kw


def _assemble(results):
    output = np.empty((T, HID), dtype=np.float32)
    hT = np.empty((B, H, D, D), dtype=np.float32)
    for b in range(B):
        c0 = b * TPD
        output[b * S:(b + 1) * S] = results[c0]["out"] + results[c0 + 1]["out"]
        for tp in range(TPD):
            ht = results[c0 + tp]["ht"]  # [d, h, e]
            hT[b, tp * NH:(tp + 1) * NH] = ht.transpose(1, 0, 2)
    return output, hT


def kernel(positions, hidden_states, recurrent_state, w_qkv, w_g, w_dense,
           q_norm_w, k_norm_w, g_norm_w):
    global LAST_EXEC_NS
    from concourse.bass_utils import run_bass_kernel_spmd

    in_maps, apply_qw, apply_kw = _make_in_maps(
        positions, hidden_states, recurrent_state, w_qkv, w_g, w_dense,
        q_norm_w, k_norm_w, g_norm_w,
    )
    nc = _get_program(apply_qw, apply_kw)
    res = run_bass_kernel_spmd(
        nc, in_maps, core_ids=list(range(NCORES)), trace=TRACE
    )
    LAST_EXEC_NS = res.exec_time_ns
    return _assemble(res.results)
